# revision 1
# baseline (speedup 1.0000x reference)
"""FAME-GCN Trainium2 kernel.

Computes, for merged adjacency final_A = sum_k w_k A_k + (sum_k w_k A_k)^T:
    U1 = final_A @ (feature @ W3) + b3
    U2 = final_A2 @ (feature @ W1) + b1
    out = concat(U1, U2, axis=1)          # [5000, 32]

Distribution: node rows sharded 625/core across 8 NeuronCores.

Per core, stripe-major (5 stripes of 125 rows):
  - the 12 relation stripes are loaded as full-row dma_gathers spread over 4
    SWDGE queues (the only DGE path that sustains >300 GB/s here),
  - temp = sum_k w_k A_k[stripe] is formed by vector/pool-engine FMA chains
    (keeps the tensor engine free),
  - dir1 = temp^T-contribution: S_own-stationary matmuls per column block,
    accumulated into [16, N] in SBUF across stripes,
  - dir2 = temp @ S: PE transposes of the merged stripe feeding an
    S-stationary accumulation chain over all column chunks.
The [16, N] dir1 partials are summed across cores on the host (the
reduce-scatter step of the row-sharded spmm); biases, final transpose and the
concat also happen on host.
"""

import sys

if "/opt/trn_rl_repo" not in sys.path:
    sys.path.insert(0, "/opt/trn_rl_repo")

import numpy as np

import concourse.bacc as bacc
import concourse.mybir as mybir
from concourse.tile import TileContext
from concourse.bass_utils import run_bass_kernel_spmd

F32 = mybir.dt.float32
F32R = mybir.dt.float32r
I16 = mybir.dt.int16
MUL = mybir.AluOpType.mult
ADD = mybir.AluOpType.add

N = 5000
NP = 5120  # padded row length (row stride must be a multiple of 256 bytes)
NFEAT = 128
OUT = 16
K_A, K_AT = 3, 9
NCORES = 8
RS = N // NCORES  # 625 rows per core
STRIPE = 125
NSTRIPE = RS // STRIPE
CB = 512
NCB = (N + CB - 1) // CB
HALF = NP // 2  # 2560: gather half-width

_CACHE = {}


def _c_blocks():
    return [(cb * CB, min(CB, N - cb * CB)) for cb in range(NCB)]


def _c_subs_all():
    subs = []
    j = 0
    while j * 128 < N:
        subs.append((j, min(128, N - j * 128)))
        j += 1
    return subs  # 40 chunks on the global 128-grid


def build():
    nc = bacc.Bacc(num_swdge_queues=4)

    a = nc.declare_dram_parameter("a", [K_A, RS, NP], F32, isOutput=False)
    at = nc.declare_dram_parameter("at", [K_AT, RS, NP], F32, isOutput=False)
    idxs = nc.declare_dram_parameter("idxs", [128, 8 * NSTRIPE], I16, isOutput=False)
    wcol = nc.declare_dram_parameter("wcol", [128, K_A + K_AT], F32, isOutput=False)
    feat = nc.declare_dram_parameter("feat", [N, NFEAT], F32, isOutput=False)
    featow = nc.declare_dram_parameter("featow", [RS, NFEAT], F32, isOutput=False)
    w3 = nc.declare_dram_parameter("w3", [NFEAT, OUT], F32, isOutput=False)
    w1 = nc.declare_dram_parameter("w1", [NFEAT, OUT], F32, isOutput=False)
    ident = nc.declare_dram_parameter("ident", [128, 128], F32R, isOutput=False)
    ident32 = nc.declare_dram_parameter("ident32", [128, 128], F32, isOutput=False)

    o1a = nc.declare_dram_parameter("o1a", [OUT, N], F32, isOutput=True)
    o1b = nc.declare_dram_parameter("o1b", [OUT, N], F32, isOutput=True)
    o2a = nc.declare_dram_parameter("o2a", [OUT, RS], F32, isOutput=True)
    o2b = nc.declare_dram_parameter("o2b", [OUT, RS], F32, isOutput=True)

    nchunks = (N + 127) // 128  # 40

    with TileContext(nc) as tc:
        with (
            tc.tile_pool(name="persist", bufs=1) as pp,
            tc.tile_pool(name="pm", bufs=2, space="PSUM") as pmp,
            tc.tile_pool(name="pt", bufs=2, space="PSUM") as ptp,
            tc.tile_pool(name="pd1", bufs=2, space="PSUM") as pd1p,
            tc.tile_pool(name="pd2", bufs=2, space="PSUM") as pd2p,
        ):
            # ---------------- persistent tiles ----------------
            w3t = pp.tile([NFEAT, OUT], F32, tag="w3t")
            w1t = pp.tile([NFEAT, OUT], F32, tag="w1t")
            nc.sync.dma_start(out=w3t, in_=w3[:, :])
            nc.sync.dma_start(out=w1t, in_=w1[:, :])
            id_t = pp.tile([128, 128], F32R, tag="ident")
            nc.sync.dma_start(out=id_t, in_=ident[:, :])
            id32_t = pp.tile([128, 128], F32, tag="ident32")
            nc.sync.dma_start(out=id32_t, in_=ident32[:, :])
            ix = pp.tile([128, 8 * NSTRIPE], I16, tag="ix")
            nc.sync.dma_start(out=ix, in_=idxs[:, :])
            wc = pp.tile([128, K_A + K_AT], F32, tag="wc")
            nc.sync.dma_start(out=wc, in_=wcol[:, :])

            s3f = pp.tile([128, nchunks * OUT], F32R, tag="s3f")
            s1f = pp.tile([128, nchunks * OUT], F32R, tag="s1f")
            s3o = pp.tile([STRIPE, NSTRIPE * OUT], F32R, tag="s3o")
            s1o = pp.tile([STRIPE, NSTRIPE * OUT], F32R, tag="s1o")

            o1sb_a = pp.tile([OUT, N], F32, tag="o1sb_a")
            o1sb_b = pp.tile([OUT, N], F32, tag="o1sb_b")
            acc2a = pp.tile([OUT, RS], F32, tag="acc2a")
            acc2b = pp.tile([OUT, RS], F32, tag="acc2b")

            # ---------------- preamble: S matrices ----------------
            with tc.tile_pool(name="pre", bufs=2) as prep:
                featT = prep.tile([NFEAT, N], F32, tag="featT", bufs=1)
                featTow = prep.tile([NFEAT, RS], F32, tag="featTow", bufs=1)
                for t in range(nchunks):
                    r0 = t * 128
                    wt = min(128, N - r0)
                    ft = prep.tile([128, NFEAT], F32, tag="ftile", name=f"ft_{t}")
                    nc.sync.dma_start(out=ft[:wt, :], in_=feat[r0 : r0 + wt, :])
                    ptr = pmp.tile([128, 128], F32, tag="pm", name=f"ptf_{t}")
                    nc.tensor.transpose(ptr[:, :wt], ft[:wt, :], id32_t[:wt, :wt])
                    nc.vector.tensor_copy(out=featT[:, r0 : r0 + wt], in_=ptr[:, :wt])
                for u in range(NSTRIPE):
                    r0 = u * STRIPE
                    ft = prep.tile([128, NFEAT], F32, tag="ftile", name=f"fto_{u}")
                    nc.sync.dma_start(
                        out=ft[:STRIPE, :], in_=featow[r0 : r0 + STRIPE, :]
                    )
                    ptr = pmp.tile([128, 128], F32, tag="pm", name=f"ptfo_{u}")
                    nc.tensor.transpose(
                        ptr[:, :STRIPE], ft[:STRIPE, :], id32_t[:STRIPE, :STRIPE]
                    )
                    nc.vector.tensor_copy(
                        out=featTow[:, r0 : r0 + STRIPE], in_=ptr[:, :STRIPE]
                    )
                for t in range(nchunks):
                    r0 = t * 128
                    wt = min(128, N - r0)
                    for wi, (wtile, sdst) in enumerate(((w3t, s3f), (w1t, s1f))):
                        ps = pmp.tile([128, OUT], F32, tag="pm", name=f"ps_{t}_{wi}")
                        nc.tensor.matmul(
                            ps[:wt, :],
                            featT[:, r0 : r0 + wt],
                            wtile,
                            start=True,
                            stop=True,
                        )
                        nc.scalar.copy(
                            out=sdst[:wt, t * OUT : (t + 1) * OUT], in_=ps[:wt, :]
                        )
                for u in range(NSTRIPE):
                    r0 = u * STRIPE
                    for wi, (wtile, sdst) in enumerate(((w3t, s3o), (w1t, s1o))):
                        ps = pmp.tile([128, OUT], F32, tag="pm", name=f"pso_{u}_{wi}")
                        nc.tensor.matmul(
                            ps[:STRIPE, :],
                            featTow[:, r0 : r0 + STRIPE],
                            wtile,
                            start=True,
                            stop=True,
                        )
                        nc.scalar.copy(
                            out=sdst[:, u * OUT : (u + 1) * OUT], in_=ps[:STRIPE, :]
                        )

            # ---------------- main loop: stripe-major ----------------
            with (
                tc.tile_pool(name="raw", bufs=6) as rawp,
                tc.tile_pool(name="mrg", bufs=1) as mrgp,
                tc.tile_pool(name="ttp", bufs=4) as ttp,
            ):
                groups = (
                    ("a", s3o, s3f, acc2a, o1sb_a),
                    ("b", s1o, s1f, acc2b, o1sb_b),
                )
                for st in range(NSTRIPE):
                    ixs = ix[:, st * 8 : (st + 1) * 8]
                    # gather the 12 relation stripes in half-rows, 4 queues
                    th = {}
                    for k in range(K_A + K_AT):
                        src = a[k, :, :] if k < K_A else at[k - K_A, :, :]
                        for h in range(2):
                            t = rawp.tile(
                                [128, 1, HALF], F32, tag="traw",
                                name=f"t_{st}_{k}_{h}",
                            )
                            nc.gpsimd.dma_gather(
                                t,
                                src[:, h * HALF : (h + 1) * HALF],
                                ixs,
                                128,
                                128,
                                HALF,
                                elem_step=NP,
                                queue_num=(2 * k + h) % 4,
                            )
                            th[(k, h)] = t
                    # merge: mrg_g = sum_k w_k * stripe_k   (vector engines)
                    mrga = mrgp.tile([128, N], F32R, tag="mrga", name=f"mrga_{st}")
                    mrgb = mrgp.tile([128, N], F32R, tag="mrgb", name=f"mrgb_{st}")
                    for h in range(2):
                        c0, cwid = (0, HALF) if h == 0 else (HALF, N - HALF)
                        msl = slice(c0, c0 + cwid)
                        tsl = slice(0, cwid)

                        def tk(k):
                            return th[(k, h)][:, 0, tsl]

                        # DVE FMA chains: mrg_g = sum_k w_k t_k
                        nc.vector.tensor_scalar_mul(mrga[:, msl], tk(0), wc[:, 0:1])
                        for k in (1, 2):
                            nc.vector.scalar_tensor_tensor(
                                mrga[:, msl], tk(k), wc[:, k : k + 1],
                                mrga[:, msl], MUL, ADD,
                            )
                        nc.vector.tensor_scalar_mul(mrgb[:, msl], tk(3), wc[:, 3:4])
                        for k in (4, 5, 6, 7, 8, 9, 10, 11):
                            nc.vector.scalar_tensor_tensor(
                                mrgb[:, msl], tk(k), wc[:, k : k + 1],
                                mrgb[:, msl], MUL, ADD,
                            )

                    for gname, so, sf, acc2, o1sb in groups:
                        mrg = mrga if gname == "a" else mrgb
                        # dir1: o1sb[:, blk] (+)= S_own[st]^T @ mrg[:, blk]
                        for cb, (c0, cw) in enumerate(_c_blocks()):
                            pd1 = pd1p.tile(
                                [OUT, CB], F32, tag="pd1",
                                name=f"pd1_{st}_{gname}_{cb}",
                            )
                            nc.tensor.matmul(
                                pd1[:, :cw],
                                so[:, st * OUT : (st + 1) * OUT],
                                mrg[:STRIPE, c0 : c0 + cw],
                                start=True,
                                stop=True,
                            )
                            dst = o1sb[:, c0 : c0 + cw]
                            if st == 0:
                                nc.vector.tensor_copy(out=dst, in_=pd1[:, :cw])
                            else:
                                nc.vector.tensor_add(dst, dst, pd1[:, :cw])
                        # dir2: acc2[:, st] = sum_j S[c_j]^T @ (mrg[:, c_j])^T
                        pd2 = pd2p.tile(
                            [OUT, 128], F32, tag="pd2", name=f"pd2_{st}_{gname}"
                        )
                        allsubs = _c_subs_all()
                        for j, cjw in allsubs:
                            ptr = ptp.tile(
                                [128, 128], F32R, tag="pt",
                                name=f"pt_{st}_{gname}_{j}",
                            )
                            nc.tensor.transpose(
                                ptr[:cjw, :126],
                                mrg[:STRIPE, 128 * j : 128 * j + cjw],
                                id_t[:STRIPE, :126],
                            )
                            strip = ttp.tile(
                                [128, 126], F32R, tag="tt",
                                name=f"tt_{st}_{gname}_{j}",
                            )
                            nc.scalar.copy(out=strip[:cjw, :], in_=ptr[:cjw, :126])
                            nc.tensor.matmul(
                                pd2[:, :126],
                                sf[:cjw, j * OUT : (j + 1) * OUT],
                                strip[:cjw, :],
                                start=(j == 0),
                                stop=(j == len(allsubs) - 1),
                            )
                        nc.vector.tensor_copy(
                            out=acc2[:, st * STRIPE : (st + 1) * STRIPE],
                            in_=pd2[:, :STRIPE],
                        )

            nc.sync.dma_start(out=o1a[:, :], in_=o1sb_a)
            nc.sync.dma_start(out=o1b[:, :], in_=o1sb_b)
            nc.sync.dma_start(out=o2a[:, :], in_=acc2a)
            nc.sync.dma_start(out=o2b[:, :], in_=acc2b)

    nc.compile()
    return nc


def _make_inputs(feature, A, A_t, w2, wb, W3, W1):
    eye = np.eye(128, dtype=np.float32)
    wvals = np.concatenate([w2, wb]).astype(np.float32)
    wcol = np.tile(wvals[None, :], (128, 1))
    idxs = np.full((128, 8 * NSTRIPE), -1, dtype=np.int16)
    for st in range(NSTRIPE):
        for j in range(STRIPE):
            for rep in range(8):
                idxs[j % 16 + 16 * rep, st * 8 + j // 16] = STRIPE * st + j

    apad = np.zeros((K_A, N, NP), dtype=np.float32)
    apad[:, :, :N] = A
    atpad = np.zeros((K_AT, N, NP), dtype=np.float32)
    atpad[:, :, :N] = A_t

    in_maps = []
    for p in range(NCORES):
        r0 = p * RS
        in_maps.append(
            {
                "a": np.ascontiguousarray(apad[:, r0 : r0 + RS, :]),
                "at": np.ascontiguousarray(atpad[:, r0 : r0 + RS, :]),
                "idxs": idxs,
                "wcol": wcol,
                "feat": feature,
                "featow": np.ascontiguousarray(feature[r0 : r0 + RS, :]),
                "w3": W3,
                "w1": W1,
                "ident": eye,
                "ident32": eye,
            }
        )
    return in_maps


def kernel(feature, A, A_t, weight_b2, weight_b, W3, b3, W1, b1, **kw):
    feature = np.asarray(feature, dtype=np.float32)
    A = np.asarray(A, dtype=np.float32)
    A_t = np.asarray(A_t, dtype=np.float32)
    w2 = np.asarray(weight_b2, dtype=np.float32).reshape(K_A)
    wb = np.asarray(weight_b, dtype=np.float32).reshape(K_AT)
    W3 = np.asarray(W3, dtype=np.float32)
    W1 = np.asarray(W1, dtype=np.float32)
    b3 = np.asarray(b3, dtype=np.float32)
    b1 = np.asarray(b1, dtype=np.float32)

    if "nc" not in _CACHE:
        _CACHE["nc"] = build()
    nc = _CACHE["nc"]

    in_maps = _make_inputs(feature, A, A_t, w2, wb, W3, W1)
    _CACHE["in_maps"] = in_maps

    res = run_bass_kernel_spmd(nc, in_maps, core_ids=list(range(NCORES)))

    col_a = np.zeros((OUT, N), dtype=np.float32)
    col_b = np.zeros((OUT, N), dtype=np.float32)
    row_a = np.empty((OUT, N), dtype=np.float32)
    row_b = np.empty((OUT, N), dtype=np.float32)
    for p in range(NCORES):
        r = res.results[p]
        col_a += r["o1a"]
        col_b += r["o1b"]
        row_a[:, p * RS : (p + 1) * RS] = r["o2a"]
        row_b[:, p * RS : (p + 1) * RS] = r["o2b"]

    U1 = (col_a + row_a).T + b3
    U2 = (col_b + row_b).T + b1
    return np.concatenate([U1, U2], axis=1).astype(np.float32)

